# revision 8
# baseline (speedup 1.0000x reference)
"""MoE layer (top-2 routing, 8 experts) for Trainium2 across 8 NeuronCores.

Strategy: pair-sharded expert parallelism in bf16, software-pipelined.
  - Gate (x @ Wg, top-2 + softmax) on host (~0.03% of FLOPs). Experts are
    paired hot+cold by routed-token count; each pair is served by 2 cores
    with both experts' FFNs split along the hidden dim H (each core holds
    W1[:, h*2048:(h+1)*2048] / W2[h*2048:...] for BOTH experts -- 16 MB,
    same SBUF footprint as one full expert). Both cores of a pair process
    the union of the pair's routed tokens over their half-FFN; host sums
    the two partial outputs.
  - SOFTWARE PIPELINING: phase A (x@W1+gelu) of chunk c+1 interleaves with
    phase B (h@W2 * gate) of chunk c in an [A,A,B]x8 pattern, so the PE
    never sits at a bulk-synchronous phase boundary.
  - DMA RESTRUCTURE (v2): everything is packed on host into flat
    [128, N] layouts, job-major with 512-column padded jobs, so each
    logical transfer is ONE contiguous DMA (8 KB/partition descriptors):
    5 xt fetches + 8 w1 + 8 w2 + 2 b1 + 1 g + 10 y-halves = 34 DMAs vs
    179 per-tile DMAs before. Issue is spread across sequencers (weights
    on Pool/SWDGE, xt+g on SP, y stores on DVE) so no sequencer
    serializes the schedule (the old kernel spent 268 us of a 275 us
    dispatch issuing DMAs back-to-back on SP). Pad columns carry gate=0
    so their outputs are exactly zero; the host combine never reads them.
  - y partials are stored bf16 (their sum is reconstructed in f32 on
    host; adds ~4e-3 rel err, well under the 2e-2 gate).
  - Phase B has NO activation op (vector engine multiplies PSUM by the
    gate directly; b2 folded into the host combine -- exact algebra), so
    gelu is the only ACT table ever loaded.
"""

import os
from contextlib import ExitStack

import ml_dtypes
import numpy as np

import concourse.bass as bass
import concourse.tile as tile
from concourse import bacc, mybir
from concourse.bass_utils import run_bass_kernel_spmd

try:  # pragma: no cover
    import antenv.axon_hooks  # noqa: F401
except ImportError:
    os.environ.setdefault("BASS_NEVER_TRACE", "1")

BF16 = ml_dtypes.bfloat16
D, H, O, E, TOPK = 1024, 4096, 1024, 8, 2
P = 128
N_CORES = 8
H2 = H // 2
N_D, N_H2, N_O = D // P, H2 // P, O // P  # 8, 16, 8
JC = 512  # padded job width (columns)

_CACHE: dict[tuple, bass.Bass] = {}


def _token_tiles(C):
    """Near-equal moving-dim chunks <= JC."""
    n_chunks = -(-C // JC)
    base, rem = divmod(C, n_chunks)
    tiles, t0 = [], 0
    for i in range(n_chunks):
        n = base + (1 if i < rem else 0)
        tiles.append((t0, n))
        t0 += n
    return tiles


def _jobs(A_cap, B_cap):
    return ([("a", t0, nt) for (t0, nt) in _token_tiles(A_cap)]
            + [("b", t0, nt) for (t0, nt) in _token_tiles(B_cap)])


def _build(A_cap: int, B_cap: int, iters: int = 1) -> bass.Bass:
    f32, bf16 = mybir.dt.float32, mybir.dt.bfloat16
    jobs = _jobs(A_cap, B_cap)
    nj = len(jobs)
    CJ = nj * JC
    nc = bacc.Bacc("TRN2", target_bir_lowering=False, debug=False,
                   num_devices=N_CORES)
    # All flat [128, N] bf16/f32, packed host-side (see _prepare):
    #   xt: job-major [p, ji, d, t(512)];  w1: [p, cc(4), d(8), c'(512)]
    #   w2: [p, h(16), c(1024)];  g: job-major [p, ji, t];  yt: [p, ji, o, t]
    xt_d = nc.dram_tensor("xt", [P, nj * N_D * JC], bf16,
                          kind="ExternalInput").ap()
    w1a_d = nc.dram_tensor("w1a", [P, N_D * H2], bf16, kind="ExternalInput").ap()
    w1b_d = nc.dram_tensor("w1b", [P, N_D * H2], bf16, kind="ExternalInput").ap()
    w2a_d = nc.dram_tensor("w2a", [P, N_H2 * O], bf16, kind="ExternalInput").ap()
    w2b_d = nc.dram_tensor("w2b", [P, N_H2 * O], bf16, kind="ExternalInput").ap()
    b1a_d = nc.dram_tensor("b1a", [P, N_H2], f32, kind="ExternalInput").ap()
    b1b_d = nc.dram_tensor("b1b", [P, N_H2], f32, kind="ExternalInput").ap()
    g_d = nc.dram_tensor("g", [P, CJ], f32, kind="ExternalInput").ap()
    yt_d = nc.dram_tensor("yt", [P, nj * N_O * JC], bf16,
                          kind="ExternalOutput").ap()

    XB = N_D * JC     # xt/yt columns per job block (4096)
    WB = N_D * JC     # w1 columns per 512-col chunk block (4096)

    with tile.TileContext(nc) as tc, ExitStack() as ctx:
        wpool = ctx.enter_context(tc.tile_pool(name="weights", bufs=1))
        xpool = ctx.enter_context(tc.tile_pool(name="xin", bufs=1))
        hpool = ctx.enter_context(tc.tile_pool(name="hts", bufs=34))
        ppool1 = ctx.enter_context(tc.tile_pool(name="ps1", bufs=3, space="PSUM"))
        ppool2 = ctx.enter_context(tc.tile_pool(name="ps2", bufs=3, space="PSUM"))
        ypool = ctx.enter_context(tc.tile_pool(name="yout", bufs=2))

        w1_sb = {"a": wpool.tile([P, N_D * H2], bf16, name="w1a"),
                 "b": wpool.tile([P, N_D * H2], bf16, name="w1b")}
        w2_sb = {"a": wpool.tile([P, N_H2 * O], bf16, name="w2a"),
                 "b": wpool.tile([P, N_H2 * O], bf16, name="w2b")}
        b1_sb = {"a": wpool.tile([P, N_H2], f32, name="b1a"),
                 "b": wpool.tile([P, N_H2], f32, name="b1b")}
        g_sb = wpool.tile([P, CJ], f32)
        w1_dram = {"a": w1a_d, "b": w1b_d}
        w2_dram = {"a": w2a_d, "b": w2b_d}
        b1_dram = {"a": b1a_d, "b": b1b_d}

        RING = 3
        xt_tiles = [xpool.tile([P, XB], bf16, name=f"xt{r}")
                    for r in range(RING)]

        def fetch(ji):
            nc.sync.dma_start(out=xt_tiles[ji % RING][:, :],
                              in_=xt_d[:, ji * XB:(ji + 1) * XB])

        def load_w(ex):
            for cc in range(H2 // JC):  # 4 chunks of 512 w1-columns
                nc.gpsimd.dma_start(
                    out=w1_sb[ex][:, cc * WB:(cc + 1) * WB],
                    in_=w1_dram[ex][:, cc * WB:(cc + 1) * WB])
            nc.gpsimd.dma_start(out=b1_sb[ex][:], in_=b1_dram[ex][:])
            for hh in range(4):  # 4 chunks of 4 h-tiles
                nc.gpsimd.dma_start(
                    out=w2_sb[ex][:, hh * 4096:(hh + 1) * 4096],
                    in_=w2_dram[ex][:, hh * 4096:(hh + 1) * 4096])

        # Cold-start order: the first A-phase needs xt0 + w1a chunk 0 only.
        fetch(0)
        nc.gpsimd.dma_start(out=w1_sb["a"][:, 0:WB], in_=w1a_d[:, 0:WB])
        nc.gpsimd.dma_start(out=b1_sb["a"][:], in_=b1a_d[:])
        fetch(1)
        for cc in range(1, H2 // JC):
            nc.gpsimd.dma_start(out=w1_sb["a"][:, cc * WB:(cc + 1) * WB],
                                in_=w1a_d[:, cc * WB:(cc + 1) * WB])
        for hh in range(4):
            nc.gpsimd.dma_start(out=w2_sb["a"][:, hh * 4096:(hh + 1) * 4096],
                                in_=w2a_d[:, hh * 4096:(hh + 1) * 4096])
        nc.sync.dma_start(out=g_sb[:], in_=g_d[:])
        fetch(2)
        load_w("b")

        gelu = mybir.ActivationFunctionType.Gelu

        loop_ctx = ExitStack()
        if iters > 1:
            loop_ctx.enter_context(tc.For_i(0, iters, 1))
        ctx.enter_context(loop_ctx)

        def a_group(ji, m):
            (ex, t0, nt) = jobs[ji]
            ps = ppool1.tile([P, JC], f32, tag="ps1")
            base = (m // 4) * WB + (m % 4) * P
            xt_t = xt_tiles[ji % RING]
            for d in range(N_D):
                nc.tensor.matmul(ps[:, :nt],
                                 lhsT=w1_sb[ex][:, base + d * JC:
                                                base + d * JC + P],
                                 rhs=xt_t[:, d * JC:d * JC + nt],
                                 start=(d == 0), stop=(d == N_D - 1))
            ht = hpool.tile([P, JC], bf16, tag="ht")
            nc.scalar.activation(ht[:, :nt], ps[:, :nt], gelu,
                                 bias=b1_sb[ex][:, m:m + 1])
            return ht

        def b_group(ji, o, hts, ym):
            (ex, t0, nt) = jobs[ji]
            ps2 = ppool2.tile([P, JC], f32, tag="ps2")
            for h in range(N_H2):
                nc.tensor.matmul(ps2[:, :nt],
                                 lhsT=w2_sb[ex][:, h * O + o * P:
                                                h * O + o * P + P],
                                 rhs=hts[h][:, :nt],
                                 start=(h == 0), stop=(h == N_H2 - 1))
            nc.vector.tensor_mul(ym[:, (o % 4) * JC:(o % 4) * JC + nt],
                                 ps2[:, :nt], g_sb[:, ji * JC:ji * JC + nt])

        def y_flush(ji, half, ym):
            off = ji * N_O * JC + half * 4 * JC
            nc.sync.dma_start(out=yt_d[:, off:off + 4 * JC], in_=ym[:, :])

        def b_block(c, hts_cur, interleave):
            # Last chunk: flush y per-o so the final store drains sooner.
            last = interleave is None
            ym = None
            for o in range(N_O):
                if o % 4 == 0:
                    ym = ypool.tile([P, 4 * JC], bf16, tag="ym")
                if interleave is not None:
                    interleave(o)
                b_group(c, o, hts_cur, ym)
                if last:
                    off = c * N_O * JC + o * JC
                    nc.sync.dma_start(out=yt_d[:, off:off + JC],
                                      in_=ym[:, (o % 4) * JC:(o % 4 + 1) * JC])
                elif o % 4 == 3:
                    y_flush(c, o // 4, ym)

        # prologue: phase A of job 0
        hts_cur = [a_group(0, m) for m in range(N_H2)]
        hts_next = []
        for c in range(nj):
            nxt = c + RING
            if nxt < nj:
                fetch(nxt)
            if iters > 1:
                for w_ in range(RING):
                    if w_ + RING * ((nj - 1 - w_) // RING) == c:
                        fetch(w_)
            if c + 1 < nj:
                hts_next = []

                def interleave(o, _c=c):
                    hts_next.append(a_group(_c + 1, 2 * o))
                    hts_next.append(a_group(_c + 1, 2 * o + 1))

                b_block(c, hts_cur, interleave)
                hts_cur = hts_next
            else:
                b_block(c, hts_cur, None)
    nc.compile()
    return nc


def _prepare(x, Wg, W1, b1, W2, b2):
    """Host gating + pair assignment + packed per-core input maps."""
    x = np.asarray(x)
    B, S, Dx = x.shape
    assert Dx == D and Wg.shape == (D, E)
    T = B * S
    xf = np.ascontiguousarray(x.reshape(T, D), dtype=np.float32)
    logits = xf.astype(np.float64) @ np.asarray(Wg, np.float64)
    top_i = np.argpartition(-logits, TOPK - 1, axis=1)[:, :TOPK]
    lv = np.take_along_axis(logits, top_i, axis=1)
    lv -= lv.max(axis=1, keepdims=True)
    ex_ = np.exp(lv)
    w = ex_ / ex_.sum(axis=1, keepdims=True)

    flat_e = top_i.reshape(-1)
    flat_w = w.reshape(-1)
    counts = np.bincount(flat_e, minlength=E)

    order = np.argsort(-counts, kind="stable")
    pairs = [(int(order[i]), int(order[E - 1 - i])) for i in range(E // 2)]
    A_cap = max(1024, int(max(counts[a] for a, _ in pairs)))
    B_cap = max(512, int(max(counts[b] for _, b in pairs)))

    jobs = _jobs(A_cap, B_cap)
    nj = len(jobs)
    CJ = nj * JC
    nja = len(_token_tiles(A_cap))
    # cap-index -> padded job-grid column
    colmap = {"a": np.empty(A_cap, np.int64), "b": np.empty(B_cap, np.int64)}
    for ji, (ex, t0, nt) in enumerate(jobs):
        colmap[ex][t0:t0 + nt] = ji * JC + np.arange(nt)

    xt_bf = np.ascontiguousarray(xf.T).astype(BF16)  # [D, T]
    W1b_ = np.asarray(W1).astype(BF16)
    W2b_ = np.asarray(W2).astype(BF16)

    def pack_feat(a, groups):  # [groups*128, M] -> [128, groups*M] grp-major
        M = a.shape[1]
        return np.ascontiguousarray(
            a.reshape(groups, P, M).transpose(1, 0, 2).reshape(P, groups * M))

    in_maps = []
    glob = np.empty(2 * T, dtype=np.int64)  # pair-slot -> row in stacked Y
    for p, (ea, eb) in enumerate(pairs):
        xt_p = np.zeros((D, CJ), dtype=BF16)
        g_p = np.zeros((CJ,), dtype=np.float32)
        for side, e in (("a", ea), ("b", eb)):
            sel = np.nonzero(flat_e == e)[0]
            tok = sel >> 1
            cols = colmap[side][:len(sel)]
            xt_p[:, cols] = xt_bf[:, tok]
            g_p[cols] = flat_w[sel]
            glob[sel] = p * CJ + cols
        # [128, nj*8*512] job-major, d-major inside each job
        xt_pk = np.ascontiguousarray(
            xt_p.reshape(N_D, P, nj, JC).transpose(1, 2, 0, 3)
            .reshape(P, nj * N_D * JC))
        g_bc = np.ascontiguousarray(np.broadcast_to(g_p, (P, CJ)))
        for hf in range(2):
            sl1 = slice(hf * H2, (hf + 1) * H2)
            # w1 half [D, H2] -> [128, cc(4), d(8), 512]
            in_maps.append({
                "xt": xt_pk,
                "w1a": np.ascontiguousarray(
                    W1b_[ea][:, sl1].reshape(N_D, P, 4, JC)
                    .transpose(1, 2, 0, 3).reshape(P, N_D * H2)),
                "w1b": np.ascontiguousarray(
                    W1b_[eb][:, sl1].reshape(N_D, P, 4, JC)
                    .transpose(1, 2, 0, 3).reshape(P, N_D * H2)),
                "w2a": pack_feat(W2b_[ea][sl1, :], N_H2),
                "w2b": pack_feat(W2b_[eb][sl1, :], N_H2),
                "b1a": np.ascontiguousarray(
                    np.asarray(b1[ea][sl1], np.float32).reshape(N_H2, P).T),
                "b1b": np.ascontiguousarray(
                    np.asarray(b1[eb][sl1], np.float32).reshape(N_H2, P).T),
                "g": g_bc,
            })
    b2f = np.asarray(b2, np.float32)
    corr = (w[:, 0:1] * b2f[top_i[:, 0]]
            + w[:, 1:2] * b2f[top_i[:, 1]]).astype(np.float32)  # [T, O]
    return in_maps, (glob, corr), (A_cap, B_cap), B, S


def _get_nc(caps, iters: int = 1) -> bass.Bass:
    key = (caps, iters)
    nc = _CACHE.get(key)
    if nc is None:
        nc = _CACHE[key] = _build(caps[0], caps[1], iters)
    return nc


def _combine(results, glob_corr, caps, B, S):
    glob, corr = glob_corr
    nj = len(_jobs(caps[0], caps[1]))
    CJ = nj * JC
    # yt [128, nj*8*512] -> [col(nj*512), feat(1024)]; sum pair halves in f32.
    # Pad columns hold stale SBUF bits (can overflow f32 in the sum) but are
    # never indexed by glob; suppress the spurious warning.
    np.seterr(over="ignore", invalid="ignore")
    Y = np.stack([
        np.asarray(results[2 * p]["yt"]).reshape(P, nj, N_O, JC)
        .transpose(1, 3, 2, 0).reshape(CJ, O).astype(np.float32)
        + np.asarray(results[2 * p + 1]["yt"]).reshape(P, nj, N_O, JC)
        .transpose(1, 3, 2, 0).reshape(CJ, O).astype(np.float32)
        for p in range(E // 2)])  # [4, CJ, O]
    Yflat = Y.reshape(4 * CJ, O)
    out = Yflat[glob[0::2]] + Yflat[glob[1::2]] + corr
    return out.reshape(B, S, O).astype(np.float32, copy=False)


def kernel(x, Wg, W1, b1, W2, b2):
    in_maps, glob, caps, B, S = _prepare(x, Wg, W1, b1, W2, b2)
    nc = _get_nc(caps)
    res = run_bass_kernel_spmd(nc, in_maps, core_ids=list(range(N_CORES)))
    return _combine(res.results, glob, caps, B, S)


# revision 13
# speedup vs baseline: 1.1463x; 1.1463x over previous
"""MoE layer (top-2, 8 experts) on 8 NeuronCores — eighth-H sharding.

Every core holds an H/8 = 512 slice of ALL 8 experts' FFNs (16 MB, same
SBUF as one full expert) and processes ALL 8192 routed token-expert
columns over its slice; the host sums the 8 per-core partials. This is
perfectly load-balanced (no caps, no pairing): 8192 x 64 = 524,288 PE
column-cycles per core, the exact ideal, vs 540,416 for the pair-sharded
half-H scheme (-3.0%). The compiled program depends only on the global
routed-token counts (identical on every core -> SPMD-clean); per-core
variation lives entirely in the host-packed weight slices.

Jobs are per-expert 512-column tiles of the concatenated routed-token
list. DMA layout is flat job-major [128, nj*4096] with 512-padded jobs
(gate=0 on pads nulls their outputs). Phase A (x@W1+gelu, 4 m-tiles) of
chunk c+1 interleaves with phase B (h@W2*g, 8 o-tiles) of chunk c as
[A,B,B]x4. Weights load once on the Pool/SWDGE queue; xt rides a 3-deep
ring on SP; y-halves flush on SP as bf16.
"""

import os
from contextlib import ExitStack

import ml_dtypes
import numpy as np

import concourse.bass as bass
import concourse.tile as tile
from concourse import bacc, mybir
from concourse.bass_utils import run_bass_kernel_spmd

try:  # pragma: no cover
    import antenv.axon_hooks  # noqa: F401
except ImportError:
    os.environ.setdefault("BASS_NEVER_TRACE", "1")

BF16 = ml_dtypes.bfloat16
D, H, O, E, TOPK = 1024, 4096, 1024, 8, 2
P = 128
N_CORES = 8
HS = H // N_CORES           # 512: per-core H slice
N_D, N_HS, N_O = D // P, HS // P, O // P  # 8, 4, 8
JC = 512

_CACHE: dict[tuple, bass.Bass] = {}


def _token_tiles(C):
    if C <= 0:
        return []
    n_chunks = -(-C // JC)
    base, rem = divmod(C, n_chunks)
    tiles, t0 = [], 0
    for i in range(n_chunks):
        n = base + (1 if i < rem else 0)
        tiles.append((t0, n))
        t0 += n
    return tiles


def _jobs(counts):
    jobs = []
    for e in range(E):
        jobs += [(e, t0, nt) for (t0, nt) in _token_tiles(int(counts[e]))]
    return jobs


def _build(counts: tuple, iters: int = 1) -> bass.Bass:
    f32, bf16 = mybir.dt.float32, mybir.dt.bfloat16
    jobs = _jobs(counts)
    nj = len(jobs)
    CJ = nj * JC
    XB = N_D * JC   # 4096 columns per job block (xt and yt)
    WB = N_D * JC   # 4096 columns per expert w1/w2 block
    nc = bacc.Bacc("TRN2", target_bir_lowering=False, debug=False,
                   num_devices=N_CORES)
    xt_d = nc.dram_tensor("xt", [P, nj * XB], bf16, kind="ExternalInput").ap()
    w1_d = nc.dram_tensor("w1", [P, E * WB], bf16, kind="ExternalInput").ap()
    w2_d = nc.dram_tensor("w2", [P, E * WB], bf16, kind="ExternalInput").ap()
    b1_d = nc.dram_tensor("b1", [P, E * N_HS], f32, kind="ExternalInput").ap()
    g_d = nc.dram_tensor("g", [P, CJ], bf16, kind="ExternalInput").ap()
    yt_d = nc.dram_tensor("yt", [P, nj * XB], bf16, kind="ExternalOutput").ap()

    with tile.TileContext(nc) as tc, ExitStack() as ctx:
        wpool = ctx.enter_context(tc.tile_pool(name="weights", bufs=1))
        xpool = ctx.enter_context(tc.tile_pool(name="xin", bufs=1))
        hpool = ctx.enter_context(tc.tile_pool(name="hts", bufs=10))
        ppool1 = ctx.enter_context(tc.tile_pool(name="ps1", bufs=3, space="PSUM"))
        ppool2 = ctx.enter_context(tc.tile_pool(name="ps2", bufs=4, space="PSUM"))
        ypool = ctx.enter_context(tc.tile_pool(name="yout", bufs=2))

        w1_sb = wpool.tile([P, E * WB], bf16, name="w1")
        w2_sb = wpool.tile([P, E * WB], bf16, name="w2")
        b1_sb = wpool.tile([P, E * N_HS], f32, name="b1")
        g_sb = wpool.tile([P, CJ], bf16)

        RING = 3
        xt_tiles = [xpool.tile([P, XB], bf16, name=f"xt{r}")
                    for r in range(RING)]

        def fetch(ji):
            nc.sync.dma_start(out=xt_tiles[ji % RING][:, :],
                              in_=xt_d[:, ji * XB:(ji + 1) * XB])

        # Cold start: weights stream in JOB order (w1[e] just before w2[e],
        # expert by expert) so block c never waits on a load queued behind
        # weights it needs much later; g's first half lands before B(0).
        expert_order = []
        for (e, _, _) in jobs:
            if e not in expert_order:
                expert_order.append(e)
        e0 = expert_order[0]
        hb = XB // 2
        nc.sync.dma_start(out=xt_tiles[0][:, :hb], in_=xt_d[:, :hb])
        nc.gpsimd.dma_start(out=w1_sb[:, e0 * WB:e0 * WB + hb],
                            in_=w1_d[:, e0 * WB:e0 * WB + hb])
        nc.sync.dma_start(out=xt_tiles[0][:, hb:XB], in_=xt_d[:, hb:XB])
        nc.gpsimd.dma_start(out=w1_sb[:, e0 * WB + hb:(e0 + 1) * WB],
                            in_=w1_d[:, e0 * WB + hb:(e0 + 1) * WB])
        nc.gpsimd.dma_start(out=b1_sb[:], in_=b1_d[:])
        fetch(1)
        nc.gpsimd.dma_start(out=w2_sb[:, e0 * WB:(e0 + 1) * WB],
                            in_=w2_d[:, e0 * WB:(e0 + 1) * WB])
        nc.sync.dma_start(out=g_sb[:, :CJ // 2], in_=g_d[:, :CJ // 2])
        fetch(2)
        qb = WB // 4
        for i, e in enumerate(expert_order[1:], 1):
            # Quarter the later experts' loads: the SWDGE prep rate then
            # self-throttles the weight stream to ~2/3 of DMA bandwidth,
            # so the xt ring's fetches never starve behind it.
            pieces = 1 if i == 1 else 4
            for w_dram, w_tile in ((w1_d, w1_sb), (w2_d, w2_sb)):
                for q in range(pieces):
                    sz = WB // pieces
                    off = e * WB + q * sz
                    nc.gpsimd.dma_start(out=w_tile[:, off:off + sz],
                                        in_=w_dram[:, off:off + sz])
            if i == 2:
                nc.sync.dma_start(out=g_sb[:, CJ // 2:], in_=g_d[:, CJ // 2:])

        gelu = mybir.ActivationFunctionType.Gelu

        loop_ctx = ExitStack()
        if iters > 1:
            loop_ctx.enter_context(tc.For_i(0, iters, 1))
        ctx.enter_context(loop_ctx)

        def a_group(ji, m):
            (e, t0, nt) = jobs[ji]
            ps = ppool1.tile([P, JC], f32, tag="ps1")
            xt_t = xt_tiles[ji % RING]
            for d in range(N_D):
                nc.tensor.matmul(ps[:, :nt],
                                 lhsT=w1_sb[:, e * WB + d * JC + m * P:
                                            e * WB + d * JC + m * P + P],
                                 rhs=xt_t[:, d * JC:d * JC + nt],
                                 start=(d == 0), stop=(d == N_D - 1))
            ht = hpool.tile([P, JC], bf16, tag="ht")
            nc.scalar.activation(ht[:, :nt], ps[:, :nt], gelu,
                                 bias=b1_sb[:, e * N_HS + m:e * N_HS + m + 1])
            return ht

        def b_group(ji, o, hts, ym):
            (e, t0, nt) = jobs[ji]
            ps2 = ppool2.tile([P, JC], f32, tag="ps2")
            for h in range(N_HS):
                nc.tensor.matmul(ps2[:, :nt],
                                 lhsT=w2_sb[:, e * WB + h * O + o * P:
                                            e * WB + h * O + o * P + P],
                                 rhs=hts[h][:, :nt],
                                 start=(h == 0), stop=(h == N_HS - 1))
            nc.vector.tensor_mul(ym[:, (o % 4) * JC:(o % 4) * JC + nt],
                                 ps2[:, :nt], g_sb[:, ji * JC:ji * JC + nt])

        def b_block(c, hts_cur, interleave):
            last = interleave is None
            ym = None
            for o in range(N_O):
                if o % 4 == 0:
                    ym = ypool.tile([P, 4 * JC], bf16, tag="ym")
                if interleave is not None and o % 2 == 0:
                    interleave(o // 2)
                b_group(c, o, hts_cur, ym)
                if last:
                    off = c * XB + o * JC
                    nc.sync.dma_start(out=yt_d[:, off:off + JC],
                                      in_=ym[:, (o % 4) * JC:(o % 4 + 1) * JC])
                elif o % 4 == 3:
                    off = c * XB + (o // 4) * 4 * JC
                    nc.sync.dma_start(out=yt_d[:, off:off + 4 * JC],
                                      in_=ym[:, :])

        hts_cur = [a_group(0, m) for m in range(N_HS)]
        hts_next = []
        for c in range(nj):
            nxt = c + RING
            if nxt < nj:
                fetch(nxt)
            if iters > 1:
                for w_ in range(RING):
                    if w_ + RING * ((nj - 1 - w_) // RING) == c:
                        fetch(w_)
            if c + 1 < nj:
                hts_next = []

                def interleave(i, _c=c):
                    hts_next.append(a_group(_c + 1, i))

                b_block(c, hts_cur, interleave)
                hts_cur = hts_next
            else:
                b_block(c, hts_cur, None)
    nc.compile()
    return nc


def _prepare(x, Wg, W1, b1, W2, b2):
    """Host gating + per-core packed input maps (eighth-H sharding)."""
    x = np.asarray(x)
    B, S, Dx = x.shape
    assert Dx == D and Wg.shape == (D, E)
    T = B * S
    xf = np.ascontiguousarray(x.reshape(T, D), dtype=np.float32)
    logits = xf.astype(np.float64) @ np.asarray(Wg, np.float64)
    top_i = np.argpartition(-logits, TOPK - 1, axis=1)[:, :TOPK]
    lv = np.take_along_axis(logits, top_i, axis=1)
    lv -= lv.max(axis=1, keepdims=True)
    ex_ = np.exp(lv)
    w = ex_ / ex_.sum(axis=1, keepdims=True)

    flat_e = top_i.reshape(-1)
    flat_w = w.reshape(-1)
    counts = tuple(int(c) for c in np.bincount(flat_e, minlength=E))

    jobs = _jobs(counts)
    nj = len(jobs)
    CJ = nj * JC
    # per-expert cap-index -> padded job-grid column
    colmap = {e: np.empty(counts[e], np.int64) for e in range(E)}
    for ji, (e, t0, nt) in enumerate(jobs):
        colmap[e][t0:t0 + nt] = ji * JC + np.arange(nt)

    xt_bf = np.ascontiguousarray(xf.T).astype(BF16)  # [D, T]
    W1b_ = np.asarray(W1).astype(BF16)
    W2b_ = np.asarray(W2).astype(BF16)
    b1f = np.asarray(b1, np.float32)

    xt_p = np.zeros((D, CJ), dtype=BF16)
    g_p = np.zeros((CJ,), dtype=np.float32)
    glob = np.empty(2 * T, dtype=np.int64)
    for e in range(E):
        sel = np.nonzero(flat_e == e)[0]
        cols = colmap[e]
        xt_p[:, cols] = xt_bf[:, sel >> 1]
        g_p[cols] = flat_w[sel]
        glob[sel] = cols
    xt_pk = np.ascontiguousarray(
        xt_p.reshape(N_D, P, nj, JC).transpose(1, 2, 0, 3)
        .reshape(P, nj * N_D * JC))
    g_bc = np.ascontiguousarray(np.broadcast_to(g_p, (P, CJ)).astype(BF16))

    in_maps = []
    for c in range(N_CORES):
        sl = slice(c * HS, (c + 1) * HS)
        w1c = np.concatenate([
            W1b_[e][:, sl].reshape(N_D, P, HS).transpose(1, 0, 2)
            .reshape(P, N_D * HS) for e in range(E)], axis=1)
        w2c = np.concatenate([
            W2b_[e][sl, :].reshape(N_HS, P, O).transpose(1, 0, 2)
            .reshape(P, N_HS * O) for e in range(E)], axis=1)
        b1c = np.concatenate([
            b1f[e][sl].reshape(N_HS, P).T for e in range(E)], axis=1)
        in_maps.append({
            "xt": xt_pk,
            "w1": np.ascontiguousarray(w1c),
            "w2": np.ascontiguousarray(w2c),
            "b1": np.ascontiguousarray(b1c),
            "g": g_bc,
        })
    b2f = np.asarray(b2, np.float32)
    corr = (w[:, 0:1] * b2f[top_i[:, 0]]
            + w[:, 1:2] * b2f[top_i[:, 1]]).astype(np.float32)
    return in_maps, (glob, corr), counts, B, S


def _get_nc(counts, iters: int = 1) -> bass.Bass:
    key = (counts, iters)
    nc = _CACHE.get(key)
    if nc is None:
        nc = _CACHE[key] = _build(counts, iters)
    return nc


def _combine(results, glob_corr, counts, B, S):
    glob, corr = glob_corr
    nj = len(_jobs(counts))
    CJ = nj * JC
    np.seterr(over="ignore", invalid="ignore")
    # Sum the 8 per-core partials first (pads hold junk; never indexed).
    Ysum = None
    for r in results:
        Yc = np.asarray(r["yt"]).reshape(P, nj, N_O, JC) \
            .transpose(1, 3, 2, 0).reshape(CJ, O).astype(np.float32)
        Ysum = Yc if Ysum is None else Ysum + Yc
    out = Ysum[glob[0::2]] + Ysum[glob[1::2]] + corr
    return out.reshape(B, S, O).astype(np.float32, copy=False)


def kernel(x, Wg, W1, b1, W2, b2):
    in_maps, glob, counts, B, S = _prepare(x, Wg, W1, b1, W2, b2)
    nc = _get_nc(counts)
    res = run_bass_kernel_spmd(nc, in_maps, core_ids=list(range(N_CORES)))
    return _combine(res.results, glob, counts, B, S)
